# revision 2
# baseline (speedup 1.0000x reference)
"""Trainium2 Bass kernel for nn_GResBlock (2-layer weighted-GCN residual block).

    h1 = relu(A @ x @ W1 + x @ W1_loop + b1)
    h2 = relu(A @ h1 @ W2 + h1 @ W2_loop + b2)
    out = (x + h2) * 0.5
(A = 50000^2 sparse adjacency given as an 800000-edge weighted list.)

Strategy (8 NeuronCores, SPMD — one program, per-core data):
- Vertices padded to 50176 = 8*6272 rows; core c owns dst nodes
  [c*6272, (c+1)*6272) split into 98 chunks of 64. Edges are bucketed by
  dst core, sorted by dst chunk, and split by src < 32768 (lo) / >= (hi)
  so int16 dma_gather indices stay in range (hi calls use a shifted base).
- Aggregation is reordered as (A @ x) @ W (associativity), so the gather
  table for layer 1 is x itself (bf16, rows padded to 256B) — no support
  matrix is ever materialized.
- Per chunk, each 128-edge block is one PE matmul: stationary = gathered
  src rows [128, 96], moving = a host-built block-sparse selector
  S [128, 64] (edge weights at the edge's dst lane; zero rows for padding),
  accumulating agg^T [96, 64] f32 in PSUM. Edge weights ride in S for free.
- Then psum2 = Wloop_aug^T @ src_aug (bias folded via an ones row) +
  W^T @ agg (one more matmul each) -> relu -> h^T chunk.
- Layer 1 tail: PE-transpose each h1^T chunk -> h1 rows -> internal DRAM;
  one AllGather builds the full 50176-row layer-2 gather table.
- Layer 2 tail: out^T = x^T/2 + relu(psum2 * 0.5); output returned
  transposed per core and re-assembled on the host.
- Gathers are 1024-index dma_gather calls (hardware scratch cap) spread
  round-robin over 4 SWDGE queues; explicit order deps keep Tile's DMASW
  sem-lane rotation consistent with the queue rotation.
"""
import os
import sys

import numpy as np
import ml_dtypes

try:
    import concourse.bass  # noqa: F401
except ImportError:
    sys.path.insert(0, "/opt/trn_rl_repo")

import concourse.bass as bass  # noqa: E402
import concourse.tile as tile  # noqa: E402
from concourse.tile_rust import add_dep_helper  # noqa: E402
from concourse import bacc, mybir  # noqa: E402
from concourse.library_config import mlp  # noqa: E402
from concourse.bass_utils import run_bass_kernel_spmd  # noqa: E402

bf16 = ml_dtypes.bfloat16
BF16 = mybir.dt.bfloat16
F32 = mybir.dt.float32
I16 = mybir.dt.int16

N_NODES = 50000
D = 96
NC = 8
SHARD = 6272
NPAD = NC * SHARD          # 50176
CHUNK = 64
NCHUNK = SHARD // CHUNK    # 98
HALF = 32768
ELEM = 128                 # gather element width (bf16 -> 256B)
NQ = 4                     # SWDGE queues
CALL_BLK = 8               # 128-edge blocks per gather call
CALL_IDX = 1024            # indices per gather call (hw scratch cap)


def _wrap_idx(idx):
    """[n] -> [128, n//16] int16 wrapped layout (idx i at [i%16, i//16],
    replicated across the 8 16-partition groups)."""
    n = idx.shape[0]
    w16 = idx.reshape(n // 16, 16).T.astype(np.int16)
    return np.tile(w16, (8, 1))


def _preprocess(edge_src, edge_dst, edge_weight):
    edge_src = np.asarray(edge_src).astype(np.int64)
    edge_dst = np.asarray(edge_dst).astype(np.int64)
    edge_weight = np.asarray(edge_weight).astype(np.float32)

    core_of = edge_dst // SHARD
    percore = []
    n_lo = np.zeros((NC, NCHUNK), np.int64)
    n_hi = np.zeros((NC, NCHUNK), np.int64)
    for c in range(NC):
        m = core_of == c
        s, d, w = edge_src[m], edge_dst[m], edge_weight[m]
        dl = d - c * SHARD
        ch = dl // CHUNK
        lane = dl % CHUNK
        lo = s < HALF
        order = np.lexsort((np.arange(len(s)), ~lo, ch))
        s, ch, lane, w, lo = s[order], ch[order], lane[order], w[order], lo[order]
        percore.append((s, ch, lane, w, lo))
        for k in range(NCHUNK):
            mk = ch == k
            n_lo[c, k] = np.count_nonzero(mk & lo)
            n_hi[c, k] = np.count_nonzero(mk & ~lo)

    B_lo = max(1, int(np.ceil(n_lo.max() / 128)))
    B_hi = max(1, int(np.ceil(n_hi.max() / 128)))
    NB = B_lo + B_hi

    out = []
    for c in range(NC):
        s, ch, lane, w, lo = percore[c]
        idx_lo = np.zeros((NCHUNK, B_lo * 128), np.int64)
        idx_hi = np.zeros((NCHUNK, B_hi * 128), np.int64)
        S = np.zeros((NCHUNK, 128, NB * CHUNK), np.float32)
        for k in range(NCHUNK):
            mk = ch == k
            sl, lal, wl, lol = s[mk], lane[mk], w[mk], lo[mk]
            a_s, a_l, a_w = sl[lol], lal[lol], wl[lol]
            nb = len(a_s)
            idx_lo[k, :nb] = a_s
            pos = np.arange(nb)
            S[k, pos % 128, (pos // 128) * CHUNK + a_l] = a_w
            b_s, b_l, b_w = sl[~lol], lal[~lol], wl[~lol]
            nb = len(b_s)
            idx_hi[k, :nb] = b_s - HALF
            pos = np.arange(nb)
            S[k, pos % 128, (B_lo + pos // 128) * CHUNK + b_l] = b_w

        def to_calls(idx2d, B):
            flat = idx2d.reshape(NCHUNK * B * 128)
            ncall = -(-(NCHUNK * B) // CALL_BLK)
            flat = np.concatenate([flat, np.zeros(ncall * CALL_IDX - flat.shape[0], np.int64)])
            return np.stack([_wrap_idx(flat[i * CALL_IDX:(i + 1) * CALL_IDX])
                             for i in range(ncall)])

        out.append(dict(
            idx_lo=to_calls(idx_lo, B_lo).astype(np.int16),
            idx_hi=to_calls(idx_hi, B_hi).astype(np.int16),
            S=S.astype(bf16),
        ))
    return out, B_lo, B_hi


def _make_in_maps(x, W1, W1_loop, b1, W2, W2_loop, b2, edge_weight, edge_src, edge_dst):
    pp, B_lo, B_hi = _preprocess(edge_src, edge_dst, edge_weight)
    x = np.asarray(x, np.float32)
    xtab = np.zeros((NPAD, ELEM), bf16)
    xtab[:N_NODES, :D] = x.astype(bf16)
    xpad = np.zeros((NPAD, D), np.float32)
    xpad[:N_NODES] = x
    W1a = np.concatenate([np.asarray(W1_loop, np.float32),
                          np.asarray(b1, np.float32)[None, :]], 0).astype(bf16)
    W2a = np.concatenate([np.asarray(W2_loop, np.float32),
                          np.asarray(b2, np.float32)[None, :]], 0).astype(bf16)
    in_maps = []
    for c in range(NC):
        xs = xpad[c * SHARD:(c + 1) * SHARD]
        xT_aug = np.ones((D + 1, SHARD), bf16)
        xT_aug[:D] = xs.T.astype(bf16)
        in_maps.append(dict(
            xtab=xtab,
            xT_aug=xT_aug,
            xT_half=np.ascontiguousarray(0.5 * xs.T).astype(np.float32),
            W1=np.asarray(W1, np.float32).astype(bf16),
            W2=np.asarray(W2, np.float32).astype(bf16),
            W1a=W1a, W2a=W2a,
            S=pp[c]["S"],
            idx_lo=pp[c]["idx_lo"],
            idx_hi=pp[c]["idx_hi"],
        ))
    return in_maps, B_lo, B_hi


def build_program(B_lo, B_hi, repeat=0, ag_reps=1):
    """Build the SPMD Bass program. repeat>0 wraps each gconv phase in a
    hardware For_i loop and emits the AllGather ag_reps times (timing only;
    collectives cannot sit inside hardware loops)."""
    NB = B_lo + B_hi
    NCALL_LO = -(-(NCHUNK * B_lo) // CALL_BLK)
    NCALL_HI = -(-(NCHUNK * B_hi) // CALL_BLK)
    nc = bacc.Bacc("TRN2", target_bir_lowering=False, debug=False, num_devices=NC,
                   num_swdge_queues=NQ)

    xtab = nc.dram_tensor("xtab", [NPAD, ELEM], BF16, kind="ExternalInput")
    xT_aug = nc.dram_tensor("xT_aug", [D + 1, SHARD], BF16, kind="ExternalInput")
    xT_half = nc.dram_tensor("xT_half", [D, SHARD], F32, kind="ExternalInput")
    W1 = nc.dram_tensor("W1", [D, D], BF16, kind="ExternalInput")
    W2 = nc.dram_tensor("W2", [D, D], BF16, kind="ExternalInput")
    W1a = nc.dram_tensor("W1a", [D + 1, D], BF16, kind="ExternalInput")
    W2a = nc.dram_tensor("W2a", [D + 1, D], BF16, kind="ExternalInput")
    S_d = nc.dram_tensor("S", [NCHUNK, 128, NB * CHUNK], BF16, kind="ExternalInput")
    idx_lo_d = nc.dram_tensor("idx_lo", [NCALL_LO, 128, CALL_IDX // 16], I16,
                              kind="ExternalInput")
    idx_hi_d = nc.dram_tensor("idx_hi", [NCALL_HI, 128, CALL_IDX // 16], I16,
                              kind="ExternalInput")
    outT = nc.dram_tensor("outT", [D, SHARD], F32, kind="ExternalOutput")

    with tile.TileContext(nc) as tc:
        from contextlib import ExitStack
        with ExitStack() as ctx:
            const = ctx.enter_context(tc.tile_pool(name="const", bufs=1))
            idxp = ctx.enter_context(tc.tile_pool(name="idxp", bufs=6))
            mlop = ctx.enter_context(tc.tile_pool(name="mlop", bufs=6))
            mhip = ctx.enter_context(tc.tile_pool(name="mhip", bufs=4))
            sp = ctx.enter_context(tc.tile_pool(name="sp", bufs=3))
            srcp = ctx.enter_context(tc.tile_pool(name="srcp", bufs=3))
            aggsbp = ctx.enter_context(tc.tile_pool(name="aggsbp", bufs=3))
            rowp = ctx.enter_context(tc.tile_pool(name="rowp", bufs=3))
            outp = ctx.enter_context(tc.tile_pool(name="outp", bufs=3))
            aggps = ctx.enter_context(tc.tile_pool(name="aggps", bufs=3, space="PSUM"))
            p2ps = ctx.enter_context(tc.tile_pool(name="p2ps", bufs=2, space="PSUM"))
            trps = ctx.enter_context(tc.tile_pool(name="trps", bufs=2, space="PSUM"))

            nc.gpsimd.load_library(mlp)

            ident_d = nc.inline_tensor(np.eye(D, dtype=bf16), name="ident_bf16")
            ident = const.tile([D, D], BF16)
            nc.sync.dma_start(ident[:], ident_d.ap())
            w1 = const.tile([D, D], BF16)
            nc.sync.dma_start(w1[:], W1.ap())
            w2 = const.tile([D, D], BF16)
            nc.sync.dma_start(w2[:], W2.ap())
            w1a = const.tile([D + 1, D], BF16)
            nc.sync.dma_start(w1a[:], W1a.ap())
            w2a = const.tile([D + 1, D], BF16)
            nc.sync.dma_start(w2a[:], W2a.ap())

            h1t = const.tile([D + 1, SHARD], BF16)   # persistent h1^T (+ones row)
            nc.vector.memset(h1t[D:D + 1, :], 1.0)

            state = {"gq": 0, "prev_gather": None}
            h1_local = nc.dram_tensor("h1_local", [SHARD, ELEM], BF16, kind="Internal").ap()
            h1_table = nc.dram_tensor("h1_table", [NPAD, ELEM], BF16, kind="Internal",
                                      addr_space="Shared").ap()

            def gconv(layer, table_ap, w_t, wa_t):
                lo_tiles = {}
                hi_tiles = {}

                def emit_call(tiles, idx_d, c, half):
                    it = idxp.tile([128, CALL_IDX // 16], I16, tag="it")
                    nc.sync.dma_start(it[:], idx_d.ap()[c])
                    m = (mlop if half == 0 else mhip).tile(
                        [128, CALL_BLK, ELEM], BF16, tag="m")
                    base = table_ap[0:HALF, :] if half == 0 else table_ap[HALF:NPAD, :]
                    gi = nc.gpsimd.dma_gather(m[:], base, it[:], CALL_IDX, CALL_IDX,
                                              ELEM, queue_num=state["gq"] % NQ)
                    state["gq"] += 1
                    if state["prev_gather"] is not None:
                        # Keep Pool-engine order = emission order so Tile's
                        # 8-lane DMASW sem rotation stays aligned with the
                        # 4-queue rotation (sems are queue-locked).
                        add_dep_helper(gi.ins, state["prev_gather"].ins, sync=False,
                                       reason="swdge queue/sem-lane consistency")
                    state["prev_gather"] = gi
                    tiles[c] = m

                for k in range(NCHUNK):
                    for j in range(B_lo):
                        c = (k * B_lo + j) // CALL_BLK
                        if c not in lo_tiles:
                            emit_call(lo_tiles, idx_lo_d, c, 0)
                    for j in range(B_hi):
                        c = (k * B_hi + j) // CALL_BLK
                        if c not in hi_tiles:
                            emit_call(hi_tiles, idx_hi_d, c, 1)
                    st = sp.tile([128, NB * CHUNK], BF16, tag="st")
                    nc.sync.dma_start(st[:], S_d.ap()[k])
                    agg = aggps.tile([D, CHUNK], F32, tag="agg")
                    for j in range(B_lo):
                        b = k * B_lo + j
                        nc.tensor.matmul(
                            agg[:], lo_tiles[b // CALL_BLK][:, b % CALL_BLK, 0:D],
                            st[:, j * CHUNK:(j + 1) * CHUNK],
                            start=(j == 0), stop=False, skip_group_check=True)
                    for j in range(B_hi):
                        b = k * B_hi + j
                        nc.tensor.matmul(
                            agg[:], hi_tiles[b // CALL_BLK][:, b % CALL_BLK, 0:D],
                            st[:, (B_lo + j) * CHUNK:(B_lo + j + 1) * CHUNK],
                            start=False, stop=(j == B_hi - 1), skip_group_check=True)
                    aggb = aggsbp.tile([D, CHUNK], BF16, tag="aggb")
                    nc.scalar.activation(aggb[:], agg[:],
                                         mybir.ActivationFunctionType.Copy)
                    p2 = p2ps.tile([D, CHUNK], F32, tag="p2")
                    if layer == 1:
                        src = srcp.tile([D + 1, CHUNK], BF16, tag="src")
                        nc.sync.dma_start(src[:], xT_aug.ap()[:, k * CHUNK:(k + 1) * CHUNK])
                        srcap = src[:]
                    else:
                        srcap = h1t[:, k * CHUNK:(k + 1) * CHUNK]
                    nc.tensor.matmul(p2[:], wa_t[:], srcap,
                                     start=True, stop=False, skip_group_check=True)
                    nc.tensor.matmul(p2[:], w_t[:], aggb[:],
                                     start=False, stop=True, skip_group_check=True)
                    if layer == 1:
                        hslice = h1t[0:D, k * CHUNK:(k + 1) * CHUNK]
                        nc.scalar.activation(hslice, p2[:],
                                             mybir.ActivationFunctionType.Relu)
                        trp = trps.tile([CHUNK, D], BF16, tag="trp")
                        nc.tensor.transpose(trp[:], hslice, ident[:])
                        row = rowp.tile([CHUNK, D], BF16, tag="row")
                        nc.vector.tensor_copy(row[:], trp[:])
                        nc.sync.dma_start(h1_local[k * CHUNK:(k + 1) * CHUNK, 0:D], row[:])
                    else:
                        relu = outp.tile([D, CHUNK], F32, tag="relu")
                        nc.scalar.activation(relu[:], p2[:],
                                             mybir.ActivationFunctionType.Relu, scale=0.5)
                        xh = srcp.tile([D, CHUNK], F32, tag="xh")
                        nc.sync.dma_start(xh[:], xT_half.ap()[:, k * CHUNK:(k + 1) * CHUNK])
                        ot = outp.tile([D, CHUNK], F32, tag="ot")
                        nc.vector.tensor_add(ot[:], relu[:], xh[:])
                        nc.sync.dma_start(outT.ap()[:, k * CHUNK:(k + 1) * CHUNK], ot[:])

            def allgather():
                nc.gpsimd.collective_compute(
                    "AllGather", mybir.AluOpType.bypass,
                    ins=[h1_local[:]], outs=[h1_table[:]],
                    replica_groups=[list(range(NC))],
                )

            if repeat > 0:
                with tc.For_i(0, repeat, 1):
                    gconv(1, xtab.ap(), w1, w1a)
                state["prev_gather"] = None
                for _ in range(ag_reps):
                    allgather()
                with tc.For_i(0, repeat, 1):
                    gconv(2, h1_table[:], w2, w2a)
            else:
                gconv(1, xtab.ap(), w1, w1a)
                allgather()
                gconv(2, h1_table[:], w2, w2a)

    nc.compile()
    return nc


_CACHE = {}


def kernel(**inputs):
    in_maps, B_lo, B_hi = _make_in_maps(**inputs)
    key = (B_lo, B_hi)
    if key not in _CACHE:
        _CACHE[key] = build_program(B_lo, B_hi)
    nc = _CACHE[key]
    r = run_bass_kernel_spmd(nc, in_maps, list(range(NC)))
    out = np.concatenate([r.results[c]["outT"].T for c in range(NC)], 0)[:N_NODES]
    return np.ascontiguousarray(out.astype(np.float32))
